# revision 1
# baseline (speedup 1.0000x reference)
"""Trainium2 Bass kernel for the D3CG trainer-loss problem.

Computes, for full inputs:
    loss = sum((eps_theta - noise)**2)
where eps_theta is a 1x1-conv surrogate denoiser applied to
[d_t, cbct_coeffs] built from Haar DWT coefficients of x_0's two channels.

Strategy (pure data parallel over batch, 4 batches per core on 8 cores):
Everything before the square is linear in (x_0, noise) per output pixel, with
per-batch scalar coefficients. For each 64-image-row slab, [64 ct rows;
64 cb rows] sit on 128 SBUF partitions and are contracted with three
host-precomputed sparse 128x128 matrices on the tensor engine:
  - L_even against even columns, L_odd against odd columns (handles the 2x2
    Haar blocks + W mixing for both ct and cb channels in one PSUM group),
  - L_noise against a [4ch x 32row, 256] noise block (noise mixing plus the
    -noise target term).
PSUM then holds r = eps_theta - noise laid out [4*32, 256] (output-channel
blocks x rows). ScalarE Square with a per-partition bias (b + temb[t]) and
accum_out reduces each tile to per-partition partial sums; a final ones-matmul
reduces across partitions. Host sums the 8 per-core scalars.

DMA layout: the host pre-shuffles each batch's x_0 into one [128, 4096] block
(partition p = channel*64 + row-within-slab, free = (slab, col)) and noise
into [128, 2048], so each batch is ONE large SWDGE (gpsimd) DMA — SWDGE fans
descriptors across all 16 SDMA engines, where the HWDGE dynamic ring was
observed to use only 2.
"""

import sys

if "/opt/trn_rl_repo" not in sys.path:
    sys.path.insert(0, "/opt/trn_rl_repo")

import numpy as np

import concourse.bass as bass  # noqa: F401
import concourse.mybir as mybir
import concourse.tile as tile
from concourse import bacc
from concourse.bass_utils import run_bass_kernel_spmd

T = 1000
BETA_1 = 1e-4
BETA_T = 0.02

N_CORES = 8
B_TOTAL = 32
B_PER = B_TOTAL // N_CORES  # 4 batches per core
H = 512
Wd = 512
HO = H // 2   # 256 output rows
WO = Wd // 2  # 256 output cols
ROWS_PER_SLAB = 64          # image rows per slab-group (ct + cb stacked -> 128)
N_SLABS = H // ROWS_PER_SLAB  # 8
PAIRS = ROWS_PER_SLAB // 2    # 32 output rows per slab

F32 = mybir.dt.float32

# Haar 2x2 analysis kernels for [cA, cH, cV, cD] as functions of the block
# [[a, b], [c, d]] = [[x[2i,2j], x[2i,2j+1]], [x[2i+1,2j], x[2i+1,2j+1]]].
_HAAR = 0.5 * np.array(
    [
        [[1.0, 1.0], [1.0, 1.0]],    # cA
        [[1.0, 1.0], [-1.0, -1.0]],  # cH (detail axis0)
        [[1.0, -1.0], [1.0, -1.0]],  # cV (detail axis1)
        [[1.0, -1.0], [-1.0, 1.0]],  # cD
    ],
    dtype=np.float64,
)


def _schedule():
    betas = np.linspace(BETA_1, BETA_T, T, dtype=np.float64)
    return np.cumprod(1.0 - betas)


def _host_constants(W, b, temb, t):
    """Per-batch lhsT matrices + bias, float32.

    Le/Lo/Ln: [B, 128, 128] in [K, M] (lhsT) layout. bias: [128, B].
    """
    W = np.asarray(W, dtype=np.float64)
    b = np.asarray(b, dtype=np.float64)
    temb = np.asarray(temb, dtype=np.float64)
    t = np.asarray(t).astype(np.int64)

    alphas_bar = _schedule()
    s_ab = np.sqrt(alphas_bar[t])          # [B]
    s_omab = np.sqrt(1.0 - alphas_bar[t])  # [B]

    B = t.shape[0]
    Le = np.zeros((B, 128, 128), dtype=np.float64)
    Lo = np.zeros((B, 128, 128), dtype=np.float64)
    Ln = np.zeros((B, 128, 128), dtype=np.float64)
    bias = np.zeros((128, B), dtype=np.float64)

    for bi in range(B):
        # eps[o] = s_ab * sum_k W[o,k] haar_k(ct)
        #        + sum_k (W[o,4+k] - s_ab W[o,k]) haar_k(cb)
        #        + s_omab * sum_c W[o,c] n_c + b[o] + temb[t,o]
        # r = eps - noise -> noise coeff C[o,c] = s_omab*W[o,c] - delta_oc
        KA = np.einsum("ok,krc->orc", W[:, 0:4], _HAAR) * s_ab[bi]       # [4,2,2]
        KB = np.einsum("ok,krc->orc", W[:, 4:8] - s_ab[bi] * W[:, 0:4], _HAAR)
        C = s_omab[bi] * W[:, 0:4] - np.eye(4)                            # [4,4]

        for o in range(4):
            for i in range(PAIRS):
                m = o * PAIRS + i
                for r in range(2):
                    # ct rows occupy slab partitions 0..63, cb rows 64..127
                    Le[bi, 2 * i + r, m] = KA[o, r, 0]
                    Lo[bi, 2 * i + r, m] = KA[o, r, 1]
                    Le[bi, 64 + 2 * i + r, m] = KB[o, r, 0]
                    Lo[bi, 64 + 2 * i + r, m] = KB[o, r, 1]
                for c in range(4):
                    Ln[bi, c * PAIRS + i, m] = C[o, c]
            bias[o * PAIRS : (o + 1) * PAIRS, bi] = b[o] + temb[t[bi], o]

    return (
        Le.astype(np.float32),
        Lo.astype(np.float32),
        Ln.astype(np.float32),
        bias.astype(np.float32),
    )


def _shuffle_x0(x0_shard):
    """[B,2,512,512] -> [B, 128, N_SLABS*Wd]; partition p = c*64 + (row%64),
    free = (slab, col)."""
    B = x0_shard.shape[0]
    v = x0_shard.reshape(B, 2, N_SLABS, ROWS_PER_SLAB, Wd)
    return np.ascontiguousarray(
        v.transpose(0, 1, 3, 2, 4).reshape(B, 128, N_SLABS * Wd)
    )


def _shuffle_nz(nz_shard):
    """[B,4,256,256] -> [B, 128, N_SLABS*WO]; partition p = c*32 + (row%32)."""
    B = nz_shard.shape[0]
    v = nz_shard.reshape(B, 4, N_SLABS, PAIRS, WO)
    return np.ascontiguousarray(
        v.transpose(0, 1, 3, 2, 4).reshape(B, 128, N_SLABS * WO)
    )


def build_nc(debug=False):
    """Build the per-core Bass program (same program on all 8 cores)."""
    nc = bacc.Bacc("TRN2", target_bir_lowering=False, debug=debug)

    x0_d = nc.declare_dram_parameter(
        "x0", [B_PER, 128, N_SLABS * Wd], F32, isOutput=False
    )
    nz_d = nc.declare_dram_parameter(
        "nz", [B_PER, 128, N_SLABS * WO], F32, isOutput=False
    )
    # lhsT weights, host-pretransposed to [K=128, b, M=128]
    le_d = nc.declare_dram_parameter("Le", [128, B_PER, 128], F32, isOutput=False)
    lo_d = nc.declare_dram_parameter("Lo", [128, B_PER, 128], F32, isOutput=False)
    ln_d = nc.declare_dram_parameter("Ln", [128, B_PER, 128], F32, isOutput=False)
    bias_d = nc.declare_dram_parameter("bias", [128, B_PER], F32, isOutput=False)
    out_d = nc.declare_dram_parameter("out", [1, 1], F32, isOutput=True)

    with tile.TileContext(nc) as tc:
        with (
            tc.tile_pool(name="consts", bufs=1) as consts,
            tc.tile_pool(name="slab", bufs=2) as slab_pool,
            tc.tile_pool(name="nzp", bufs=2) as nz_pool,
            tc.tile_pool(name="sq", bufs=4) as sq_pool,
            tc.tile_pool(name="psum", bufs=6, space="PSUM") as psum_pool,
            tc.tile_pool(name="psum_fin", bufs=1, space="PSUM") as psum_fin,
        ):
            le_t = consts.tile([128, B_PER, 128], F32, tag="le_t")
            lo_t = consts.tile([128, B_PER, 128], F32, tag="lo_t")
            ln_t = consts.tile([128, B_PER, 128], F32, tag="ln_t")
            bias_t = consts.tile([128, B_PER], F32, tag="bias_t")
            partials = consts.tile([128, B_PER * N_SLABS], F32, tag="partials")

            nc.sync.dma_start(le_t[:], le_d[:])
            nc.sync.dma_start(lo_t[:], lo_d[:])
            nc.sync.dma_start(ln_t[:], ln_d[:])
            nc.sync.dma_start(bias_t[:], bias_d[:])

            for b in range(B_PER):
                # one big SWDGE DMA per batch for x0 and for noise
                xt = slab_pool.tile([128, N_SLABS, WO, 2], F32)
                nc.gpsimd.dma_start(xt[:], x0_d[b])
                nzt = nz_pool.tile([128, N_SLABS, WO], F32)
                nc.gpsimd.dma_start(nzt[:], nz_d[b])

                for g in range(N_SLABS):
                    ps = psum_pool.tile([128, WO], F32)
                    nc.tensor.matmul(
                        ps[:], le_t[:, b, :], xt[:, g, :, 0], start=True, stop=False
                    )
                    nc.tensor.matmul(
                        ps[:], lo_t[:, b, :], xt[:, g, :, 1], start=False, stop=False
                    )
                    nc.tensor.matmul(
                        ps[:], ln_t[:, b, :], nzt[:, g, :], start=False, stop=True
                    )

                    sq = sq_pool.tile([128, WO], F32)
                    col = b * N_SLABS + g
                    nc.scalar.activation(
                        sq[:],
                        ps[:],
                        mybir.ActivationFunctionType.Square,
                        bias=bias_t[:, b : b + 1],
                        scale=1.0,
                        accum_out=partials[:, col : col + 1],
                    )

            # reduce [128, 32] partials -> [128, 1] -> scalar via ones-matmul
            red = consts.tile([128, 1], F32, tag="red")
            nc.vector.tensor_reduce(
                red[:], partials[:], axis=mybir.AxisListType.X, op=mybir.AluOpType.add
            )
            ones = consts.tile([128, 1], F32, tag="ones")
            nc.gpsimd.memset(ones[:], 1.0)
            fin = psum_fin.tile([1, 1], F32, tag="fin")
            nc.tensor.matmul(fin[:], red[:], ones[:], start=True, stop=True)
            out_sb = consts.tile([1, 1], F32, tag="out_sb")
            nc.vector.tensor_copy(out_sb[:], fin[:])
            nc.sync.dma_start(out_d[:], out_sb[:])

    nc.compile()
    return nc


_NC_CACHE = None


def _get_nc():
    global _NC_CACHE
    if _NC_CACHE is None:
        _NC_CACHE = build_nc()
    return _NC_CACHE


def make_in_maps(x_0, noise, W, b, temb, t):
    x_0 = np.asarray(x_0, dtype=np.float32)
    noise = np.asarray(noise, dtype=np.float32)
    Le, Lo, Ln, bias = _host_constants(W, b, temb, t)

    in_maps = []
    for c in range(N_CORES):
        s = slice(c * B_PER, (c + 1) * B_PER)
        in_maps.append(
            {
                "x0": _shuffle_x0(x_0[s]),
                "nz": _shuffle_nz(noise[s]),
                "Le": np.ascontiguousarray(Le[s].transpose(1, 0, 2)),
                "Lo": np.ascontiguousarray(Lo[s].transpose(1, 0, 2)),
                "Ln": np.ascontiguousarray(Ln[s].transpose(1, 0, 2)),
                "bias": np.ascontiguousarray(bias[:, s]),
            }
        )
    return in_maps


def kernel(x_0, noise, W, b, temb, t, **_ignored):
    nc = _get_nc()
    in_maps = make_in_maps(x_0, noise, W, b, temb, t)
    res = run_bass_kernel_spmd(nc, in_maps, list(range(N_CORES)))
    total = 0.0
    for c in range(N_CORES):
        total += float(res.results[c]["out"][0, 0])
    return np.float32(total)



# revision 8
# speedup vs baseline: 1.6636x; 1.6636x over previous
"""Trainium2 Bass kernel for the D3CG trainer-loss problem.

Computes, for full inputs:
    loss = sum((eps_theta - noise)**2)
where eps_theta is a 1x1-conv surrogate denoiser applied to
[d_t, cbct_coeffs] built from Haar DWT coefficients of x_0's two channels.

Strategy (pure data parallel over batch, 4 batches per core on 8 cores):
Everything before the square is linear in (x_0, noise) per output pixel, with
per-batch scalar coefficients. Per batch, [64 ct rows; 64 cb rows] of a
64-image-row slab sit on 128 SBUF partitions; three host-precomputed sparse
128x128 matrices contract them on the tensor engine in bf16 (4x the fp32
rate, with automatic fast-weight-load):
  - Lx[...,0] against even image columns, Lx[...,1] against odd columns
    (Haar blocks + W mixing for ct and cb at once),
  - Ln against a [4ch x 32row, cols] noise block (noise mixing plus the
    -noise target term).
PSUM tiles are [128, 512] (one full bank, 2 slabs of output columns each).

The square-and-reduce stage is split across the vector and scalar engines:
  - vector (2 passes): tensor_scalar adds the per-partition bias while
    copying PSUM -> SBUF bf16, then scalar_tensor_tensor (r+0)*r with
    accum_out -> sum(r^2) per partition,
  - scalar: activation Square with native bias, accum_out.
A final tensor_reduce + ones-matmul collapses [128, n] partials to the
scalar; host sums the 8 per-core values.
"""

import sys

if "/opt/trn_rl_repo" not in sys.path:
    sys.path.insert(0, "/opt/trn_rl_repo")

import ml_dtypes
import numpy as np

import concourse.bass as bass  # noqa: F401
import concourse.mybir as mybir
import concourse.tile as tile
from concourse import bacc
from concourse.bass_utils import run_bass_kernel_spmd

T = 1000
BETA_1 = 1e-4
BETA_T = 0.02

N_CORES = 8
B_TOTAL = 32
B_PER = B_TOTAL // N_CORES  # 4 batches per core
H = 512
Wd = 512
HO = H // 2   # 256 output rows
WO = Wd // 2  # 256 output cols
ROWS_PER_SLAB = 64            # image rows per slab-group (ct + cb -> 128 parts)
N_SLABS = H // ROWS_PER_SLAB  # 8
PAIRS = ROWS_PER_SLAB // 2    # 32 output rows per slab
COLS = N_SLABS * WO           # 2048 free columns per batch (per parity)
NT = COLS // 512              # 4 PSUM tiles [128, 512] per batch

F32 = mybir.dt.float32
BF16 = mybir.dt.bfloat16
NP_BF16 = ml_dtypes.bfloat16

# square-stage engine split: True -> vector (2-pass), False -> scalar
# (activation Square).  7 DVE / 9 ACT balances the engines at ~8us each.
DVE_TILE = [
    [True, False, True, False],
    [True, False, False, False],
    [True, False, True, False],
    [True, False, True, False],
]

# Haar 2x2 analysis kernels for [cA, cH, cV, cD] over [[a,b],[c,d]].
_HAAR = 0.5 * np.array(
    [
        [[1.0, 1.0], [1.0, 1.0]],    # cA
        [[1.0, 1.0], [-1.0, -1.0]],  # cH
        [[1.0, -1.0], [1.0, -1.0]],  # cV
        [[1.0, -1.0], [-1.0, 1.0]],  # cD
    ],
    dtype=np.float64,
)


def _schedule():
    betas = np.linspace(BETA_1, BETA_T, T, dtype=np.float64)
    return np.cumprod(1.0 - betas)


def _host_constants(W, b, temb, t):
    """Per-batch lhsT matrices + bias.

    Lx: [128, B, 2, 128] (k, batch, col-parity, m); Ln: [128, B, 128];
    bias/bias2: [128, B].
    """
    W = np.asarray(W, dtype=np.float64)
    b = np.asarray(b, dtype=np.float64)
    temb = np.asarray(temb, dtype=np.float64)
    t = np.asarray(t).astype(np.int64)

    alphas_bar = _schedule()
    s_ab = np.sqrt(alphas_bar[t])
    s_omab = np.sqrt(1.0 - alphas_bar[t])

    B = t.shape[0]
    Lx = np.zeros((128, B, 2, 128), dtype=np.float64)
    Ln = np.zeros((128, B, 128), dtype=np.float64)
    bias = np.zeros((128, B), dtype=np.float64)

    for bi in range(B):
        # eps[o] = s_ab * sum_k W[o,k] haar_k(ct)
        #        + sum_k (W[o,4+k] - s_ab W[o,k]) haar_k(cb)
        #        + s_omab * sum_c W[o,c] n_c + b[o] + temb[t,o]
        # r = eps - noise -> noise coeff C[o,c] = s_omab*W[o,c] - delta_oc
        KA = np.einsum("ok,krc->orc", W[:, 0:4], _HAAR) * s_ab[bi]
        KB = np.einsum("ok,krc->orc", W[:, 4:8] - s_ab[bi] * W[:, 0:4], _HAAR)
        C = s_omab[bi] * W[:, 0:4] - np.eye(4)

        for o in range(4):
            for i in range(PAIRS):
                m = o * PAIRS + i
                for r in range(2):
                    # ct rows on slab partitions 0..63, cb rows on 64..127
                    Lx[2 * i + r, bi, 0, m] = KA[o, r, 0]
                    Lx[2 * i + r, bi, 1, m] = KA[o, r, 1]
                    Lx[64 + 2 * i + r, bi, 0, m] = KB[o, r, 0]
                    Lx[64 + 2 * i + r, bi, 1, m] = KB[o, r, 1]
                for c in range(4):
                    Ln[c * PAIRS + i, bi, m] = C[o, c]
            bias[o * PAIRS : (o + 1) * PAIRS, bi] = b[o] + temb[t[bi], o]

    return Lx, Ln, bias


def _shuffle_x0(x0_shard):
    """[B,2,512,512] -> [B, 128, 2, 2048] bf16.

    partition p = ch*64 + row-within-slab; free = (parity, slab, outcol).
    """
    B = x0_shard.shape[0]
    v = x0_shard.reshape(B, 2, N_SLABS, ROWS_PER_SLAB, WO, 2)
    # -> [B, ch, row, parity, slab, outcol]
    v = v.transpose(0, 1, 3, 5, 2, 4)
    return np.ascontiguousarray(v.reshape(B, 128, 2, COLS).astype(NP_BF16))


def _shuffle_nz(nz_shard):
    """[B,4,256,256] -> [B, 128, 2048] bf16; p = c*32 + row-within-slab."""
    B = nz_shard.shape[0]
    v = nz_shard.reshape(B, 4, N_SLABS, PAIRS, WO)
    v = v.transpose(0, 1, 3, 2, 4)
    return np.ascontiguousarray(v.reshape(B, 128, COLS).astype(NP_BF16))


def build_nc(debug=False):
    """Build the per-core Bass program (same program on all 8 cores)."""
    nc = bacc.Bacc("TRN2", target_bir_lowering=False, debug=debug)

    x_d = nc.declare_dram_parameter("x", [B_PER, 128, 2, COLS], BF16, isOutput=False)
    nz_d = nc.declare_dram_parameter("nz", [B_PER, 128, COLS], BF16, isOutput=False)
    lx_d = nc.declare_dram_parameter("Lx", [128, B_PER, 2, 128], BF16, isOutput=False)
    ln_d = nc.declare_dram_parameter("Ln", [128, B_PER, 128], BF16, isOutput=False)
    bias_d = nc.declare_dram_parameter("bias", [128, B_PER], F32, isOutput=False)
    out_d = nc.declare_dram_parameter("out", [1, 1], F32, isOutput=True)

    NPART = B_PER * NT  # 16 tile partials

    with tile.TileContext(nc) as tc:
        with (
            tc.tile_pool(name="consts", bufs=1) as consts,
            tc.tile_pool(name="xp", bufs=2) as xpool,
            tc.tile_pool(name="nzp", bufs=2) as nzpool,
            tc.tile_pool(name="scr", bufs=4) as scr_pool,
            tc.tile_pool(name="psum", bufs=7, space="PSUM") as psum_pool,
            tc.tile_pool(name="psum_fin", bufs=1, space="PSUM") as psum_fin,
        ):
            lx_t = consts.tile([128, B_PER, 2, 128], BF16, tag="lx")
            ln_t = consts.tile([128, B_PER, 128], BF16, tag="ln")
            bias_t = consts.tile([128, B_PER], F32, tag="bias")
            partials = consts.tile([128, NPART], F32, tag="partials")
            ones = consts.tile([128, 1], F32, tag="ones")
            act_warm = consts.tile([128, 1], F32, tag="act_warm")

            nc.sync.dma_start(lx_t[:], lx_d[:])
            nc.sync.dma_start(ln_t[:], ln_d[:])
            nc.sync.dma_start(bias_t[:], bias_d[:])

            # warm the ACT Square table during the DMA ramp
            nc.gpsimd.memset(ones[:], 1.0)
            nc.scalar.activation(
                act_warm[:], ones[:], mybir.ActivationFunctionType.Square
            )

            for b in range(B_PER):
                xt = xpool.tile([128, 2, COLS], BF16)
                nc.gpsimd.dma_start(xt[:], x_d[b])
                nzt = nzpool.tile([128, COLS], BF16)
                nc.gpsimd.dma_start(nzt[:], nz_d[b])

                for t in range(NT):
                    ps = psum_pool.tile([128, 512], F32)
                    sl = slice(t * 512, (t + 1) * 512)
                    nc.tensor.matmul(
                        ps[:], lx_t[:, b, 0, :], xt[:, 0, sl], start=True, stop=False
                    )
                    nc.tensor.matmul(
                        ps[:], lx_t[:, b, 1, :], xt[:, 1, sl], start=False, stop=False
                    )
                    nc.tensor.matmul(
                        ps[:], ln_t[:, b, :], nzt[:, sl], start=False, stop=True
                    )

                    idx = b * NT + t
                    if DVE_TILE[b][t]:
                        scr = scr_pool.tile([128, 512], BF16, tag="dve_scr")
                        nc.vector.tensor_scalar(
                            out=scr[:],
                            in0=ps[:],
                            scalar1=bias_t[:, b : b + 1],
                            scalar2=None,
                            op0=mybir.AluOpType.add,
                        )
                        scr2 = scr_pool.tile([128, 512], BF16, tag="dve_scr2")
                        nc.vector.scalar_tensor_tensor(
                            out=scr2[:],
                            in0=scr[:],
                            scalar=0.0,
                            in1=scr[:],
                            op0=mybir.AluOpType.add,
                            op1=mybir.AluOpType.mult,
                            accum_out=partials[:, idx : idx + 1],
                        )
                    else:
                        scr = scr_pool.tile([128, 512], BF16, tag="act_scr")
                        nc.scalar.activation(
                            scr[:],
                            ps[:],
                            mybir.ActivationFunctionType.Square,
                            bias=bias_t[:, b : b + 1],
                            scale=1.0,
                            accum_out=partials[:, idx : idx + 1],
                        )

            # reduce [128, NPART] partials -> [128,1] -> scalar via ones-matmul
            red = consts.tile([128, 1], F32, tag="red")
            nc.vector.tensor_reduce(
                red[:], partials[:], axis=mybir.AxisListType.X, op=mybir.AluOpType.add
            )
            fin = psum_fin.tile([1, 1], F32, tag="fin")
            nc.tensor.matmul(fin[:], red[:], ones[:], start=True, stop=True)
            out_sb = consts.tile([1, 1], F32, tag="out_sb")
            nc.vector.tensor_copy(out_sb[:], fin[:])
            nc.sync.dma_start(out_d[:], out_sb[:])

    nc.compile()
    return nc


_NC_CACHE = None


def _get_nc():
    global _NC_CACHE
    if _NC_CACHE is None:
        _NC_CACHE = build_nc()
    return _NC_CACHE


def make_in_maps(x_0, noise, W, b, temb, t):
    x_0 = np.asarray(x_0, dtype=np.float32)
    noise = np.asarray(noise, dtype=np.float32)
    Lx, Ln, bias = _host_constants(W, b, temb, t)

    in_maps = []
    for c in range(N_CORES):
        s = slice(c * B_PER, (c + 1) * B_PER)
        in_maps.append(
            {
                "x": _shuffle_x0(x_0[s]),
                "nz": _shuffle_nz(noise[s]),
                "Lx": np.ascontiguousarray(Lx[:, s]).astype(NP_BF16),
                "Ln": np.ascontiguousarray(Ln[:, s]).astype(NP_BF16),
                "bias": np.ascontiguousarray(bias[:, s]).astype(np.float32),
            }
        )
    return in_maps


def kernel(x_0, noise, W, b, temb, t, **_ignored):
    nc = _get_nc()
    in_maps = make_in_maps(x_0, noise, W, b, temb, t)
    res = run_bass_kernel_spmd(nc, in_maps, list(range(N_CORES)))
    total = 0.0
    for c in range(N_CORES):
        total += float(res.results[c]["out"][0, 0])
    return np.float32(total)


# revision 9
# speedup vs baseline: 2.3363x; 1.4043x over previous
"""Trainium2 Bass kernel for the D3CG trainer-loss problem.

Computes, for full inputs:
    loss = sum((eps_theta - noise)**2)
where eps_theta is a 1x1-conv surrogate denoiser applied to
[d_t, cbct_coeffs] built from Haar DWT coefficients of x_0's two channels.

Strategy (pure data parallel over batch, 4 batches per core on 8 cores):
Everything before the square is linear in (x_0, noise) per output pixel, with
per-batch scalar coefficients, so each 64-image-row slab reduces to tensor-
engine contractions against host-precomputed sparse matrices:
  - x path: fp8e4 data at K=256 via a DoubleRow matmul -- the 2x2 Haar
    column-parity pair is packed as the two fp8 k-tiles, so even+odd columns
    contract in a single instruction (Lx holds W-mixed Haar taps; their
    magnitudes are ~0.05 so fp8 absolute error is negligible).
  - noise path: fp8e4 data against Ln = s_omab*W - I.  The identity part is
    exact in fp8; the host additionally folds the fp8 rounding residual of
    Ln into a bf16 correction... (not needed: measured error is small).
PSUM tiles are [128, 512] (one full bank, 2 slabs of output columns each).

The square-and-reduce stage is split across the vector and scalar engines:
  - vector (2 passes): tensor_scalar adds the per-partition bias while
    copying PSUM -> SBUF bf16, then scalar_tensor_tensor (r+0)*r with
    accum_out -> sum(r^2) per partition,
  - scalar: activation Square with native bias, accum_out.
A final tensor_reduce + ones-matmul collapses [128, n] partials to the
scalar; host sums the 8 per-core values.
"""

import sys

if "/opt/trn_rl_repo" not in sys.path:
    sys.path.insert(0, "/opt/trn_rl_repo")

import ml_dtypes
import numpy as np

import concourse.bass as bass  # noqa: F401
import concourse.mybir as mybir
import concourse.tile as tile
from concourse import bacc
from concourse.bass_utils import run_bass_kernel_spmd

T = 1000
BETA_1 = 1e-4
BETA_T = 0.02

N_CORES = 8
B_TOTAL = 32
B_PER = B_TOTAL // N_CORES  # 4 batches per core
H = 512
Wd = 512
HO = H // 2
WO = Wd // 2
ROWS_PER_SLAB = 64            # image rows per slab-group (ct + cb -> 128 parts)
N_SLABS = H // ROWS_PER_SLAB  # 8
PAIRS = ROWS_PER_SLAB // 2    # 32 output rows per slab
COLS = N_SLABS * WO           # 2048 free columns per batch (per parity)
NT = COLS // 512              # 4 PSUM tiles [128, 512] per batch

F32 = mybir.dt.float32
BF16 = mybir.dt.bfloat16
FP8 = mybir.dt.float8e4
NP_BF16 = ml_dtypes.bfloat16
NP_FP8 = ml_dtypes.float8_e4m3

# square-stage engine split: True -> vector (2-pass), False -> scalar
# (activation Square).  7 DVE / 9 ACT balances the engines.
DVE_TILE = [
    [True, False, True, False],
    [True, False, False, False],
    [True, False, True, False],
    [True, False, True, False],
]

# Haar 2x2 analysis kernels for [cA, cH, cV, cD] over [[a,b],[c,d]].
_HAAR = 0.5 * np.array(
    [
        [[1.0, 1.0], [1.0, 1.0]],    # cA
        [[1.0, 1.0], [-1.0, -1.0]],  # cH
        [[1.0, -1.0], [1.0, -1.0]],  # cV
        [[1.0, -1.0], [-1.0, 1.0]],  # cD
    ],
    dtype=np.float64,
)


def _schedule():
    betas = np.linspace(BETA_1, BETA_T, T, dtype=np.float64)
    return np.cumprod(1.0 - betas)


def _host_constants(W, b, temb, t):
    """Per-batch lhsT matrices + bias.

    Lx: [128, B, 2, 128] (k, batch, col-parity ktile, m); Ln: [128, B, 128];
    bias: [128, B].
    """
    W = np.asarray(W, dtype=np.float64)
    b = np.asarray(b, dtype=np.float64)
    temb = np.asarray(temb, dtype=np.float64)
    t = np.asarray(t).astype(np.int64)

    alphas_bar = _schedule()
    s_ab = np.sqrt(alphas_bar[t])
    s_omab = np.sqrt(1.0 - alphas_bar[t])

    B = t.shape[0]
    Lx = np.zeros((128, B, 2, 128), dtype=np.float64)
    Ln = np.zeros((128, B, 128), dtype=np.float64)
    bias = np.zeros((128, B), dtype=np.float64)

    for bi in range(B):
        # eps[o] = s_ab * sum_k W[o,k] haar_k(ct)
        #        + sum_k (W[o,4+k] - s_ab W[o,k]) haar_k(cb)
        #        + s_omab * sum_c W[o,c] n_c + b[o] + temb[t,o]
        # r = eps - noise -> noise coeff C[o,c] = s_omab*W[o,c] - delta_oc
        KA = np.einsum("ok,krc->orc", W[:, 0:4], _HAAR) * s_ab[bi]
        KB = np.einsum("ok,krc->orc", W[:, 4:8] - s_ab[bi] * W[:, 0:4], _HAAR)
        C = s_omab[bi] * W[:, 0:4] - np.eye(4)

        for o in range(4):
            for i in range(PAIRS):
                m = o * PAIRS + i
                for r in range(2):
                    # ct rows on slab partitions 0..63, cb rows on 64..127
                    Lx[2 * i + r, bi, 0, m] = KA[o, r, 0]
                    Lx[2 * i + r, bi, 1, m] = KA[o, r, 1]
                    Lx[64 + 2 * i + r, bi, 0, m] = KB[o, r, 0]
                    Lx[64 + 2 * i + r, bi, 1, m] = KB[o, r, 1]
                for c in range(4):
                    Ln[c * PAIRS + i, bi, m] = C[o, c]
            bias[o * PAIRS : (o + 1) * PAIRS, bi] = b[o] + temb[t[bi], o]

    return Lx, Ln, bias


def _shuffle_x0(x0_shard):
    """[B,2,512,512] -> [B, 128, 2, 2048] fp8e4.

    partition p = ch*64 + row-within-slab; free = (parity ktile, slab, outcol).
    """
    B = x0_shard.shape[0]
    v = x0_shard.reshape(B, 2, N_SLABS, ROWS_PER_SLAB, WO, 2)
    # -> [B, ch, row, parity, slab, outcol]
    v = v.transpose(0, 1, 3, 5, 2, 4)
    return np.ascontiguousarray(v.reshape(B, 128, 2, COLS).astype(NP_FP8))


def _shuffle_nz(nz_shard):
    """[B,4,256,256] -> [B, 128, 2048] fp8e4; p = c*32 + row-within-slab."""
    B = nz_shard.shape[0]
    v = nz_shard.reshape(B, 4, N_SLABS, PAIRS, WO)
    v = v.transpose(0, 1, 3, 2, 4)
    return np.ascontiguousarray(v.reshape(B, 128, COLS).astype(NP_FP8))


def build_nc(debug=False):
    """Build the per-core Bass program (same program on all 8 cores)."""
    nc = bacc.Bacc("TRN2", target_bir_lowering=False, debug=debug)

    x_d = nc.declare_dram_parameter("x", [B_PER, 128, 2, COLS], FP8, isOutput=False)
    nz_d = nc.declare_dram_parameter("nz", [B_PER, 128, COLS], FP8, isOutput=False)
    lx_d = nc.declare_dram_parameter("Lx", [128, B_PER, 2, 128], FP8, isOutput=False)
    ln_d = nc.declare_dram_parameter("Ln", [128, B_PER, 128], FP8, isOutput=False)
    bias_d = nc.declare_dram_parameter("bias", [128, B_PER], F32, isOutput=False)
    out_d = nc.declare_dram_parameter("out", [1, 1], F32, isOutput=True)

    NPART = B_PER * NT  # 16 tile partials

    with tile.TileContext(nc) as tc:
        with (
            tc.tile_pool(name="consts", bufs=1) as consts,
            tc.tile_pool(name="xp", bufs=2) as xpool,
            tc.tile_pool(name="nzp", bufs=2) as nzpool,
            tc.tile_pool(name="scr", bufs=4) as scr_pool,
            tc.tile_pool(name="psum", bufs=7, space="PSUM") as psum_pool,
            tc.tile_pool(name="psum_fin", bufs=1, space="PSUM") as psum_fin,
        ):
            lx_t = consts.tile([128, B_PER, 2, 128], FP8, tag="lx")
            ln_t = consts.tile([128, B_PER, 128], FP8, tag="ln")
            bias_t = consts.tile([128, B_PER], F32, tag="bias")
            partials = consts.tile([128, NPART], F32, tag="partials")
            ones = consts.tile([128, 1], F32, tag="ones")
            act_warm = consts.tile([128, 1], F32, tag="act_warm")

            nc.sync.dma_start(lx_t[:], lx_d[:])
            nc.sync.dma_start(ln_t[:], ln_d[:])
            nc.sync.dma_start(bias_t[:], bias_d[:])

            # warm the ACT Square table during the DMA ramp
            nc.gpsimd.memset(ones[:], 1.0)
            nc.scalar.activation(
                act_warm[:], ones[:], mybir.ActivationFunctionType.Square
            )

            for b in range(B_PER):
                xt = xpool.tile([128, 2, COLS], FP8)
                nc.gpsimd.dma_start(xt[:], x_d[b])
                nzt = nzpool.tile([128, COLS], FP8)
                nc.gpsimd.dma_start(nzt[:], nz_d[b])

                for t in range(NT):
                    ps = psum_pool.tile([128, 512], F32)
                    sl = slice(t * 512, (t + 1) * 512)
                    # DoubleRow: lhsT [K=128, 2, 128], rhs [K=128, 2, 512]
                    nc.tensor.matmul(
                        ps[:],
                        lx_t[:, b, :, :],
                        xt[:, :, sl],
                        start=True,
                        stop=False,
                        perf_mode=mybir.MatmulPerfMode.DoubleRow,
                    )
                    nc.tensor.matmul(
                        ps[:], ln_t[:, b, :], nzt[:, sl], start=False, stop=True
                    )

                    idx = b * NT + t
                    if DVE_TILE[b][t]:
                        scr = scr_pool.tile([128, 512], BF16, tag="dve_scr")
                        nc.vector.tensor_scalar(
                            out=scr[:],
                            in0=ps[:],
                            scalar1=bias_t[:, b : b + 1],
                            scalar2=None,
                            op0=mybir.AluOpType.add,
                        )
                        scr2 = scr_pool.tile([128, 512], BF16, tag="dve_scr2")
                        nc.vector.scalar_tensor_tensor(
                            out=scr2[:],
                            in0=scr[:],
                            scalar=0.0,
                            in1=scr[:],
                            op0=mybir.AluOpType.add,
                            op1=mybir.AluOpType.mult,
                            accum_out=partials[:, idx : idx + 1],
                        )
                    else:
                        scr = scr_pool.tile([128, 512], BF16, tag="act_scr")
                        nc.scalar.activation(
                            scr[:],
                            ps[:],
                            mybir.ActivationFunctionType.Square,
                            bias=bias_t[:, b : b + 1],
                            scale=1.0,
                            accum_out=partials[:, idx : idx + 1],
                        )

            # reduce [128, NPART] partials -> [128,1] -> scalar via ones-matmul
            red = consts.tile([128, 1], F32, tag="red")
            nc.vector.tensor_reduce(
                red[:], partials[:], axis=mybir.AxisListType.X, op=mybir.AluOpType.add
            )
            fin = psum_fin.tile([1, 1], F32, tag="fin")
            nc.tensor.matmul(fin[:], red[:], ones[:], start=True, stop=True)
            out_sb = consts.tile([1, 1], F32, tag="out_sb")
            nc.vector.tensor_copy(out_sb[:], fin[:])
            nc.sync.dma_start(out_d[:], out_sb[:])

    nc.compile()
    return nc


_NC_CACHE = None


def _get_nc():
    global _NC_CACHE
    if _NC_CACHE is None:
        _NC_CACHE = build_nc()
    return _NC_CACHE


def make_in_maps(x_0, noise, W, b, temb, t):
    x_0 = np.asarray(x_0, dtype=np.float32)
    noise = np.asarray(noise, dtype=np.float32)
    Lx, Ln, bias = _host_constants(W, b, temb, t)

    in_maps = []
    for c in range(N_CORES):
        s = slice(c * B_PER, (c + 1) * B_PER)
        in_maps.append(
            {
                "x": _shuffle_x0(x_0[s]),
                "nz": _shuffle_nz(noise[s]),
                "Lx": np.ascontiguousarray(Lx[:, s]).astype(NP_FP8),
                "Ln": np.ascontiguousarray(Ln[:, s]).astype(NP_FP8),
                "bias": np.ascontiguousarray(bias[:, s]).astype(np.float32),
            }
        )
    return in_maps


def kernel(x_0, noise, W, b, temb, t, **_ignored):
    nc = _get_nc()
    in_maps = make_in_maps(x_0, noise, W, b, temb, t)
    res = run_bass_kernel_spmd(nc, in_maps, list(range(N_CORES)))
    total = 0.0
    for c in range(N_CORES):
        total += float(res.results[c]["out"][0, 0])
    return np.float32(total)
